# revision 1
# baseline (speedup 1.0000x reference)
"""Causal self-attention (RoPE, 16 heads) Trainium2 Bass kernel.

Problem: B=8, S=1024, D=1024, H=16, HS=64, fp32, causal + all-ones padding mask.

Strategy: data-parallel over batch — one batch element per NeuronCore (8 cores).
Per-core computation uses a fully "transposed activation" layout so no on-chip
transposes are needed beyond the initial x -> x^T:

  x^T   [D, S]   via 64 PE transposes of 128x128 tiles
  Q^T,K^T [D, S] = W^T @ x^T  (lhsT = W as stored, rhs = x^T)  + RoPE fused via
                   host-precomputed coefficient tiles (deinterleaved head layout
                   by permuting W_q/W_k columns; Q additionally scaled 1/sqrt(hs))
  V     [S, D]   = x @ W_v (lhsT = x^T chunks, rhs = W_v), stored per-head with
                   an appended ones-column so att@v also yields softmax sums
  S^T   [k, q]   = (K^T)^T-chunks @ Q^T  (per head, causal blocks only)
  att^T          = exp(S^T) (no max-subtraction needed: |scores| is small),
                   diag blocks masked by a host 0/1 triangle tile
  y^T   [D, S]   accumulated per head: lhsT = [v | 1] chunk, rhs = att^T chunk;
                   row 64 gives softmax sums; normalize with reciprocal
                   broadcast to 64 partitions via a DRAM-roundtrip DMA
  out   [S, D]   = y @ W_proj (lhsT = y^T chunks, rhs = W_proj)

All matmuls run in float32r (fp32 data, fast PE mode); everything else fp32.
"""

import os

# The Bass kernel executes through the axon PJRT backend and needs the
# NeuronCores visible; a JAX_PLATFORMS=cpu pin (used for jax reference
# computation) would hide them.
if "axon" not in os.environ.get("JAX_PLATFORMS", "axon"):
    os.environ.pop("JAX_PLATFORMS", None)

import numpy as np
from contextlib import ExitStack

import concourse.bass as bass
import concourse.mybir as mybir
import concourse.tile as tile
from concourse import bacc
from concourse.bass_utils import run_bass_kernel_spmd

B, S, D, H, HS = 8, 1024, 1024, 16, 64
P = 128
NCORES = 8
F32 = mybir.dt.float32
F32R = mybir.dt.float32r
EXP = mybir.ActivationFunctionType.Exp

_CACHE = {}


def _build_nc():
    nc = bacc.Bacc(
        "TRN2", target_bir_lowering=False, debug=False, num_devices=NCORES)
    x_d = nc.dram_tensor("x", [S, D], F32R, kind="ExternalInput")
    wq_d = nc.dram_tensor("wq", [D, D], F32R, kind="ExternalInput")
    wk_d = nc.dram_tensor("wk", [D, D], F32R, kind="ExternalInput")
    wv_d = nc.dram_tensor("wv", [D, D], F32R, kind="ExternalInput")
    wp_d = nc.dram_tensor("wp", [D, D], F32R, kind="ExternalInput")
    c1q_d = nc.dram_tensor("c1q", [P, S], F32, kind="ExternalInput")
    c2q_d = nc.dram_tensor("c2q", [P, S], F32, kind="ExternalInput")
    c1k_d = nc.dram_tensor("c1k", [P, S], F32, kind="ExternalInput")
    c2k_d = nc.dram_tensor("c2k", [P, S], F32, kind="ExternalInput")
    mask_d = nc.dram_tensor("mask", [P, P], F32, kind="ExternalInput")
    ident_d = nc.dram_tensor("ident", [P, P], F32R, kind="ExternalInput")
    ones_d = nc.dram_tensor("ones", [P, H], F32, kind="ExternalInput")
    zeros_d = nc.dram_tensor("zeros", [P, 384], F32, kind="ExternalInput")
    out_d = nc.dram_tensor("out", [S, D], F32, kind="ExternalOutput")

    def mm(out, lhsT, rhs, start, stop):
        nc.tensor.matmul(out, lhsT, rhs, start=start, stop=stop)

    with tile.TileContext(nc) as tc, ExitStack() as ctx:
        persist = ctx.enter_context(tc.tile_pool(name="persist", bufs=1))
        qt = [persist.tile([P, S], F32R, name=f"qt{i}", tag=f"qt{i}") for i in range(8)]
        kt = [persist.tile([P, S], F32R, name=f"kt{i}", tag=f"kt{i}") for i in range(8)]
        vt = [persist.tile([P, H, HS + 1], F32R, name=f"vt{i}", tag=f"vt{i}")
              for i in range(8)]
        c1q = persist.tile([P, S], F32, name="c1q_t", tag="c1q_t")
        c2q = persist.tile([P, S], F32, name="c2q_t", tag="c2q_t")
        c1k = persist.tile([P, S], F32, name="c1k_t", tag="c1k_t")
        c2k = persist.tile([P, S], F32, name="c2k_t", tag="c2k_t")
        maskt = persist.tile([P, P], F32, name="maskt", tag="maskt")
        for t, d_ in ((c1q, c1q_d), (c2q, c2q_d), (c1k, c1k_d), (c2k, c2k_d),
                      (maskt, mask_d)):
            nc.sync.dma_start(t[:], d_[:])
        ident = persist.tile([P, P], F32R, name="ident", tag="ident")
        nc.sync.dma_start(ident[:], ident_d[:])
        ones_t = persist.tile([P, H], F32, name="ones_t", tag="ones_t")
        nc.sync.dma_start(ones_t[:], ones_d[:])
        zeros_t = persist.tile([P, 384], F32, name="zeros_t", tag="zeros_t")
        nc.sync.dma_start(zeros_t[:], zeros_d[:])

        # ---------------- Phase A+B: x^T, QKV, RoPE ----------------
        with ExitStack() as pctx:
            xin = pctx.enter_context(tc.tile_pool(name="xin", bufs=3))
            xtp = pctx.enter_context(tc.tile_pool(name="xtp", bufs=1))
            xt = [xtp.tile([P, S], F32R, name=f"xt{i}", tag=f"xt{i}") for i in range(8)]
            wst = pctx.enter_context(tc.tile_pool(name="wst", bufs=18))
            wvst = pctx.enter_context(tc.tile_pool(name="wvst", bufs=9))
            rtmp = pctx.enter_context(tc.tile_pool(name="rtmp", bufs=3))
            pa = pctx.enter_context(tc.tile_pool(name="pa", bufs=3, space="PSUM"))
            pb = pctx.enter_context(tc.tile_pool(name="pb", bufs=4, space="PSUM"))

            for sc in range(8):
                xtile = xin.tile([P, D], F32R, name="xtile", tag="xin")
                nc.sync.dma_start(xtile[:], x_d[sc * P:(sc + 1) * P, :])
                for dc in range(8):
                    pt = pa.tile([P, P], F32, name="pt", tag="tp")
                    nc.tensor.matmul(
                        pt[:].bitcast(F32R),
                        xtile[:, dc * P:(dc + 1) * P],
                        ident[:],
                        is_transpose=True,
                    )
                    nc.vector.tensor_copy(xt[dc][:, sc * P:(sc + 1) * P], pt[:])

            def rope(ps, dst_slice, c1, c2, s0):
                # dst = ps * c1 + swap32(ps) * c2
                t = rtmp.tile([P, 512], F32, name="ropet", tag="rt")
                for g, src in ((0, 32), (1, 0), (2, 96), (3, 64)):
                    nc.scalar.copy(t[g * 32:(g + 1) * 32, :], ps[src:src + 32, :])
                nc.vector.tensor_mul(dst_slice, ps[:], c1[:, s0:s0 + 512])
                nc.vector.tensor_mul(t[:], t[:], c2[:, s0:s0 + 512])
                nc.vector.tensor_add(dst_slice, dst_slice, t[:])

            for wd, dst, c1, c2 in ((wq_d, qt, c1q, c2q), (wk_d, kt, c1k, c2k)):
                for fc in range(8):
                    wts = []
                    for dc in range(8):
                        wtile = wst.tile([P, P], F32R, name="wtile", tag="w")
                        nc.sync.dma_start(
                            wtile[:], wd[dc * P:(dc + 1) * P, fc * P:(fc + 1) * P])
                        wts.append(wtile)
                    for s2 in range(2):
                        ps = pb.tile([P, 512], F32, name="qkps", tag="qkps")
                        for dc in range(8):
                            mm(ps[:], wts[dc][:], xt[dc][:, s2 * 512:(s2 + 1) * 512],
                               dc == 0, dc == 7)
                        rope(ps, dst[fc][:, s2 * 512:(s2 + 1) * 512], c1, c2, s2 * 512)

            for f2 in range(2):
                wvts = []
                for dc in range(8):
                    wvtile = wvst.tile([P, 512], F32R, name="wvtile", tag="wv")
                    nc.sync.dma_start(
                        wvtile[:], wv_d[dc * P:(dc + 1) * P, f2 * 512:(f2 + 1) * 512])
                    wvts.append(wvtile)
                for sc in range(8):
                    ps = pb.tile([P, 512], F32, name="vps", tag="qkps")
                    for dc in range(8):
                        mm(ps[:], xt[dc][:, sc * P:(sc + 1) * P], wvts[dc][:],
                           dc == 0, dc == 7)
                    nc.vector.tensor_copy(
                        vt[sc][:, f2 * 8:(f2 + 1) * 8, 0:HS],
                        ps[:].rearrange("p (h e) -> p h e", e=HS))
            for sc in range(8):
                nc.vector.tensor_copy(vt[sc][:, :, HS], ones_t[:])

        # ---------------- Phase C+D ----------------
        with ExitStack() as cdctx:
            ytp = cdctx.enter_context(tc.tile_pool(name="ytp", bufs=1))
            yt = [ytp.tile([P, S], F32R, name=f"yt{i}", tag=f"yt{i}") for i in range(8)]

            with ExitStack() as cctx:
                attp = cctx.enter_context(tc.tile_pool(name="attp", bufs=17))
                smallp = cctx.enter_context(tc.tile_pool(name="smallp", bufs=4))
                pss_p = cctx.enter_context(tc.tile_pool(name="pss", bufs=5, space="PSUM"))
                psy_p = cctx.enter_context(tc.tile_pool(name="psy", bufs=3, space="PSUM"))

                def score_block(ft, hb, qc, kc):
                    # scores^T block then exp (only the causally allowed span)
                    pss = pss_p.tile([P, 512], F32, name="pss", tag="pss")
                    mm(pss[:], kt[ft][hb:hb + 64, kc * P:(kc + 1) * P],
                       qt[ft][hb:hb + 64, qc * 512:(qc + 1) * 512], True, True)
                    att = attp.tile([P, 512], F32R, name="att", tag="att")
                    qsub = kc * P - qc * 512
                    if 0 <= qsub < 512:
                        if qsub > 0:
                            nc.vector.tensor_copy(att[:, 0:qsub], zeros_t[:, 0:qsub])
                        nc.scalar.activation(att[:, qsub:], pss[:, qsub:], EXP)
                        nc.vector.tensor_mul(
                            att[:, qsub:qsub + P], att[:, qsub:qsub + P], maskt[:])
                    else:
                        nc.scalar.activation(att[:], pss[:], EXP)
                    return att

                for ft in range(8):
                    for qc in range(2):
                        kmax = 4 if qc == 0 else 8
                        psyA = psy_p.tile([HS + 1, 512], F32, name="psyA", tag="psy")
                        psyB = psy_p.tile([HS + 1, 512], F32, name="psyB", tag="psy")
                        # burst all score matmuls (adjacent K=64 pairs share the
                        # PE array via row groups 0/64); exps chase on ACT
                        atts = []
                        for kc in range(kmax):
                            atts.append(score_block(ft, 0, qc, kc))
                            atts.append(score_block(ft, 64, qc, kc))
                        for kc in range(kmax):
                            mm(psyA[:], vt[kc][:, 2 * ft, :], atts[2 * kc][:],
                               kc == 0, kc == kmax - 1)
                            mm(psyB[:], vt[kc][:, 2 * ft + 1, :], atts[2 * kc + 1][:],
                               kc == 0, kc == kmax - 1)
                        for hb, psy in ((0, psyA), (64, psyB)):
                            # free the psum bank ASAP (high-priority copies),
                            # then normalize off the PE critical path
                            srow = smallp.tile([1, 512], F32, name="srow",
                                               tag="srow")
                            with tc.high_priority(offset=200):
                                nc.vector.tensor_copy(
                                    yt[ft][hb:hb + 64, qc * 512:(qc + 1) * 512],
                                    psy[0:HS, :])
                                nc.vector.tensor_copy(srow[:], psy[HS:HS + 1, :])
                            rb = smallp.tile([P, 512], F32, name="rb", tag="rb")
                            nc.gpsimd.partition_broadcast(rb[:], srow[0:1, :])
                            nc.vector.reciprocal_approx_fast(out=rb[:], in_=rb[:])
                            sl = yt[ft][hb:hb + 64, qc * 512:(qc + 1) * 512]
                            nc.vector.tensor_mul(sl, sl, rb[hb:hb + 64, :])

            with ExitStack() as dctx:
                wpst = dctx.enter_context(tc.tile_pool(name="wpst", bufs=12))
                outp = dctx.enter_context(tc.tile_pool(name="outp", bufs=4))
                psp_p = dctx.enter_context(tc.tile_pool(name="psp", bufs=3, space="PSUM"))
                for n2 in range(2):
                    wpts = []
                    for dc in range(8):
                        wptile = wpst.tile([P, 512], F32R, name="wptile", tag="wp")
                        nc.sync.dma_start(
                            wptile[:], wp_d[dc * P:(dc + 1) * P, n2 * 512:(n2 + 1) * 512])
                        wpts.append(wptile)
                    for sc in range(8):
                        psp = psp_p.tile([P, 512], F32, name="psp", tag="psp")
                        for dc in range(8):
                            mm(psp[:], yt[dc][:, sc * P:(sc + 1) * P], wpts[dc][:],
                               dc == 0, dc == 7)
                        ot = outp.tile([P, 512], F32, name="ot", tag="ot")
                        nc.scalar.copy(ot[:], psp[:])
                        nc.sync.dma_start(
                            out_d[sc * P:(sc + 1) * P, n2 * 512:(n2 + 1) * 512], ot[:])
    nc.compile()
    return nc


def _prep(inputs):
    w_qkv = np.asarray(inputs["w_qkv"], np.float32)
    w_proj = np.asarray(inputs["w_proj"], np.float32)
    cos = np.asarray(inputs["cos"], np.float32).reshape(S, HS // 2)
    sin = np.asarray(inputs["sin"], np.float32).reshape(S, HS // 2)
    wq, wk, wv = w_qkv[:, 0:D], w_qkv[:, D:2 * D], w_qkv[:, 2 * D:3 * D]
    perm = np.empty(D, np.int64)
    for h in range(H):
        b0 = h * HS
        perm[b0:b0 + HS // 2] = b0 + np.arange(0, HS, 2)
        perm[b0 + HS // 2:b0 + HS] = b0 + np.arange(1, HS, 2)
    wq, wk = wq[:, perm], wk[:, perm]
    cosT = np.ascontiguousarray(cos.T)  # [32, S]
    sinT = np.ascontiguousarray(sin.T)
    c1 = np.concatenate([cosT, cosT, cosT, cosT], 0)  # [128, S]
    c2 = np.concatenate([-sinT, sinT, -sinT, sinT], 0)
    scale = np.float32(1.0 / np.sqrt(HS))
    mask = np.triu(np.ones((P, P), np.float32))  # [k, q]: allow q >= k
    common = {
        "wq": np.ascontiguousarray(wq), "wk": np.ascontiguousarray(wk),
        "wv": np.ascontiguousarray(wv), "wp": np.ascontiguousarray(w_proj),
        "c1q": c1 * scale, "c2q": c2 * scale, "c1k": c1, "c2k": c2,
        "mask": mask, "ident": np.eye(P, dtype=np.float32),
        "ones": np.ones((P, H), np.float32), "zeros": np.zeros((P, 384), np.float32),
    }
    return common


LAST_RESULT = None


def kernel(**inputs):
    global LAST_RESULT
    if "nc" not in _CACHE:
        _CACHE["nc"] = _build_nc()
    nc = _CACHE["nc"]
    common = _prep(inputs)
    x = np.asarray(inputs["x"], np.float32)
    in_maps = [dict(common, x=np.ascontiguousarray(x[b])) for b in range(B)]
    res = run_bass_kernel_spmd(nc, in_maps, list(range(NCORES)))
    LAST_RESULT = res
    out = np.stack([res.results[i]["out"] for i in range(B)], 0)
    return out.astype(np.float32)



# revision 24
# speedup vs baseline: 1.4796x; 1.4796x over previous
"""Causal self-attention (RoPE, 16 heads) Trainium2 Bass kernel — v2 (bf16).

Problem: B=8, S=1024, D=1024, H=16, HS=64, fp32 in/out, causal mask.

Strategy: data-parallel over batch — one batch element per NeuronCore.
All matmuls run in bf16 (PE 1 cycle/col warm); fp32 accumulation in PSUM.
Rel-err tolerance is 2e-2; bf16 lands ~5e-3.

Differences vs v1 (fp32r, 453µs):
  - x is transposed on the HOST → no on-chip transpose phase.
  - bf16 weights/activations → half DMA, FWL weight loads, 2x/4x DVE modes.
  - RoPE pair-swap is an intra-32-lane stream_shuffle on DVE. The head
    deinterleave permutation pairs rope element j with j+16 inside each
    32-row group so the swap never crosses a DVE quadrant (stream_shuffle
    applies one 32-lane mask per quadrant).
  - Score/att@v matmuls and exps narrowed to the causal span.
  - Softmax reciprocal runs on the [1,512] sum row BEFORE the broadcast.
  - Per-head-pair loop interleaves QK projection (PE) with the previous
    pair's exp (ACT) / normalize (DVE) so no engine serializes the rest.
  - Causal-triangle mask multiplies run on the otherwise-idle GPSIMD.
"""

import os

if "axon" not in os.environ.get("JAX_PLATFORMS", "axon"):
    os.environ.pop("JAX_PLATFORMS", None)

import numpy as np
import ml_dtypes
from contextlib import ExitStack

import concourse.bass as bass
import concourse.mybir as mybir
import concourse.tile as tile
from concourse import bacc
from concourse.bass_utils import run_bass_kernel_spmd

B, S, D, H, HS = 8, 1024, 1024, 16, 64
P = 128
NCORES = 8
F32 = mybir.dt.float32
BF = mybir.dt.bfloat16
EXP = mybir.ActivationFunctionType.Exp
MUL = mybir.AluOpType.mult
ADD = mybir.AluOpType.add
NPBF = ml_dtypes.bfloat16

# out lane i <- in lane (i+16) % 32, same permutation in every 32-group
SHUF = [(i + 16) % 32 for i in range(32)]
USE_SHUFFLE = os.environ.get("USE_SHUFFLE", "1") == "1"
USE_STT = os.environ.get("USE_STT", "1") == "1"
USE_NARROW = os.environ.get("USE_NARROW", "1") == "1"
DEBUG_DUMP = os.environ.get("DEBUG_DUMP", "0") == "1"

_CACHE = {}


def _build_nc():
    nc = bacc.Bacc(
        "TRN2", target_bir_lowering=False, debug=False, num_devices=NCORES)
    x_d = nc.dram_tensor("x", [D, S], BF, kind="ExternalInput")  # x^T
    wq_d = nc.dram_tensor("wq", [D, D], BF, kind="ExternalInput")
    wk_d = nc.dram_tensor("wk", [D, D], BF, kind="ExternalInput")
    wv_d = nc.dram_tensor("wv", [D, D], BF, kind="ExternalInput")
    wp_d = nc.dram_tensor("wp", [D, D], BF, kind="ExternalInput")
    c1q_d = nc.dram_tensor("c1q", [P, S], BF, kind="ExternalInput")
    c2q_d = nc.dram_tensor("c2q", [P, S], BF, kind="ExternalInput")
    c1k_d = nc.dram_tensor("c1k", [P, S], BF, kind="ExternalInput")
    c2k_d = nc.dram_tensor("c2k", [P, S], BF, kind="ExternalInput")
    mask_d = nc.dram_tensor("mask", [P, P], BF, kind="ExternalInput")
    ones_d = nc.dram_tensor("ones", [P, H], BF, kind="ExternalInput")
    out_d = nc.dram_tensor("out", [S, D], F32, kind="ExternalOutput")
    if DEBUG_DUMP:
        dbg = {
            "dq": nc.dram_tensor("dq", [P, S], BF, kind="ExternalOutput"),
            "dk": nc.dram_tensor("dk", [P, S], BF, kind="ExternalOutput"),
            "dv": nc.dram_tensor("dv", [P, H * (HS + 1)], BF, kind="ExternalOutput"),
            "datt": nc.dram_tensor("datt", [P, 512], BF, kind="ExternalOutput"),
            "dpsy": nc.dram_tensor("dpsy", [HS + 1, 512], F32, kind="ExternalOutput"),
            "drb": nc.dram_tensor("drb", [P, 512], F32, kind="ExternalOutput"),
            "dy": nc.dram_tensor("dy", [P, S], BF, kind="ExternalOutput"),
            "dxt": nc.dram_tensor("dxt", [P, S], BF, kind="ExternalOutput"),
            "dwq": nc.dram_tensor("dwq", [P, D], BF, kind="ExternalOutput"),
        }

    def mm(out, lhsT, rhs, start, stop):
        nc.tensor.matmul(out, lhsT, rhs, start=start, stop=stop)

    with tile.TileContext(nc) as tc, ExitStack() as ctx:
        persist = ctx.enter_context(tc.tile_pool(name="persist", bufs=1))
        qt = [persist.tile([P, S], BF, name=f"qt{i}", tag=f"qt{i}") for i in range(8)]
        kt = [persist.tile([P, S], BF, name=f"kt{i}", tag=f"kt{i}") for i in range(8)]
        vt = [persist.tile([P, H, HS + 1], BF, name=f"vt{i}", tag=f"vt{i}")
              for i in range(8)]
        yt = [persist.tile([P, S], BF, name=f"yt{i}", tag=f"yt{i}") for i in range(8)]
        c1q = persist.tile([P, S], BF, name="c1q_t", tag="c1q_t")
        c2q = persist.tile([P, S], BF, name="c2q_t", tag="c2q_t")
        c1k = persist.tile([P, S], BF, name="c1k_t", tag="c1k_t")
        c2k = persist.tile([P, S], BF, name="c2k_t", tag="c2k_t")
        maskt = persist.tile([P, P], BF, name="maskt", tag="maskt")
        ones_t = persist.tile([P, H], BF, name="ones_t", tag="ones_t")
        for t, d_ in ((c1q, c1q_d), (c2q, c2q_d), (c1k, c1k_d), (c2k, c2k_d),
                      (maskt, mask_d), (ones_t, ones_d)):
            nc.sync.dma_start(t[:], d_[:])
        wpt = []
        for dc in range(8):
            wtile = persist.tile([P, D], BF, name=f"wpt{dc}", tag=f"wpt{dc}")
            wpt.append(wtile)

        with ExitStack() as pctx:
            xtp = pctx.enter_context(tc.tile_pool(name="xtp", bufs=1))
            wqkp = pctx.enter_context(tc.tile_pool(name="wqkp", bufs=1))
            wvp = pctx.enter_context(tc.tile_pool(name="wvp", bufs=1))
            rtmp = pctx.enter_context(tc.tile_pool(name="rtmp", bufs=4))
            attp = pctx.enter_context(tc.tile_pool(name="attp", bufs=12))
            smallp = pctx.enter_context(tc.tile_pool(name="smallp", bufs=4))
            pb = pctx.enter_context(tc.tile_pool(name="pb", bufs=2, space="PSUM"))
            pss_p = pctx.enter_context(tc.tile_pool(name="pss", bufs=3, space="PSUM"))
            psy_p = pctx.enter_context(tc.tile_pool(name="psy", bufs=3, space="PSUM"))

            # DMA order = consumption order: x, wv (V phase), wq/wk, wp.
            xt, wvt, wqt, wkt = [], [], [], []
            for dc in range(8):
                xtile = xtp.tile([P, S], BF, name=f"xt{dc}", tag=f"xt{dc}")
                nc.sync.dma_start(xtile[:], x_d[dc * P:(dc + 1) * P, :])
                xt.append(xtile)
                wtile = wvp.tile([P, D], BF, name=f"wvt{dc}", tag=f"wvt{dc}")
                nc.sync.dma_start(wtile[:], wv_d[dc * P:(dc + 1) * P, :])
                wvt.append(wtile)
            for dc in range(8):
                wtile = wqkp.tile([P, D], BF, name=f"wqt{dc}", tag=f"wqt{dc}")
                nc.sync.dma_start(wtile[:], wq_d[dc * P:(dc + 1) * P, :])
                wqt.append(wtile)
                wtile = wqkp.tile([P, D], BF, name=f"wkt{dc}", tag=f"wkt{dc}")
                nc.sync.dma_start(wtile[:], wk_d[dc * P:(dc + 1) * P, :])
                wkt.append(wtile)
            for dc in range(8):
                nc.sync.dma_start(wpt[dc][:], wp_d[dc * P:(dc + 1) * P, :])

            # ---------------- Phase 0: V = x @ wv ----------------
            for sc in range(8):
                for f2 in range(2):
                    ps = pb.tile([P, 512], F32, name="vps", tag="qkps")
                    for dc in range(8):
                        mm(ps[:], xt[dc][:, sc * P:(sc + 1) * P],
                           wvt[dc][:, f2 * 512:(f2 + 1) * 512], dc == 0, dc == 7)
                    nc.vector.tensor_copy(
                        vt[sc][:, f2 * 8:(f2 + 1) * 8, 0:HS],
                        ps[:].rearrange("p (h e) -> p h e", e=HS))
                nc.vector.tensor_copy(vt[sc][:, :, HS], ones_t[:])
                if DEBUG_DUMP and sc == 0:
                    nc.sync.dma_start(
                        dbg["dv"][:], vt[0][:].rearrange("p h e -> p (h e)"))
                    nc.sync.dma_start(dbg["dxt"][:], xt[0][:])
                    nc.sync.dma_start(dbg["dwq"][:], wqt[0][:])

            # ---------------- Phase 1: per head-pair ft ----------------
            def score_block(ft, hb, qc, kc):
                qsub = kc * P - qc * 512
                n0 = max(qsub, 0) if USE_NARROW else 0
                pss = pss_p.tile([P, 512], F32, name="pss", tag="pss")
                mm(pss[:, n0:], kt[ft][hb:hb + 64, kc * P:(kc + 1) * P],
                   qt[ft][hb:hb + 64, qc * 512 + n0:(qc + 1) * 512], True, True)
                att = attp.tile([P, 512], BF, name="att", tag="att")
                nc.scalar.activation(att[:, n0:], pss[:, n0:], EXP)
                if not USE_NARROW and qsub > 0:
                    nc.vector.memset(att[:, 0:qsub], 0)
                if 0 <= qsub < 512:
                    m0 = max(qsub, 0)
                    if USE_STT:
                        nc.vector.scalar_tensor_tensor(
                            att[:, m0:m0 + P], att[:, m0:m0 + P], 1.0,
                            maskt[:], MUL, MUL)
                    else:
                        nc.vector.tensor_mul(
                            att[:, m0:m0 + P], att[:, m0:m0 + P], maskt[:])
                return att

            for ft in range(8):
                # QK projection + rope for this head pair
                for wt, dst, c1, c2 in ((wqt, qt, c1q, c2q), (wkt, kt, c1k, c2k)):
                    for s2 in range(2):
                        sl = slice(s2 * 512, (s2 + 1) * 512)
                        ps = pb.tile([P, 512], F32, name="qkps", tag="qkps")
                        for dc in range(8):
                            mm(ps[:], wt[dc][:, ft * P:(ft + 1) * P],
                               xt[dc][:, sl], dc == 0, dc == 7)
                        # rope: dst = praw*c1 + shuffle(praw)*c2
                        praw = rtmp.tile([P, 512], BF, name="praw", tag="praw")
                        nc.vector.tensor_copy(praw[:], ps[:])
                        t = rtmp.tile([P, 512], BF, name="ropet", tag="rt")
                        if USE_SHUFFLE:
                            nc.vector.stream_shuffle(t[:], praw[:], SHUF)
                        else:
                            # ±32 block swap (32-aligned partition shifts)
                            for g, src in ((0, 32), (1, 0), (2, 96), (3, 64)):
                                nc.scalar.copy(
                                    t[g * 32:(g + 1) * 32, :],
                                    praw[src:src + 32, :])
                        t2 = rtmp.tile([P, 512], BF, name="ropet2", tag="rt2")
                        if USE_STT:
                            nc.vector.scalar_tensor_tensor(
                                dst[ft][:, sl], praw[:], 1.0, c1[:, sl], MUL, MUL)
                            nc.vector.scalar_tensor_tensor(
                                t2[:], t[:], 1.0, c2[:, sl], MUL, MUL)
                            nc.vector.scalar_tensor_tensor(
                                dst[ft][:, sl], t2[:], 1.0, dst[ft][:, sl], MUL, ADD)
                        else:
                            nc.vector.tensor_mul(dst[ft][:, sl], praw[:], c1[:, sl])
                            nc.vector.tensor_mul(t2[:], t[:], c2[:, sl])
                            nc.vector.tensor_add(
                                dst[ft][:, sl], dst[ft][:, sl], t2[:])

                if DEBUG_DUMP and ft == 0:
                    nc.sync.dma_start(dbg["dq"][:], qt[0][:])
                    nc.sync.dma_start(dbg["dk"][:], kt[0][:])
                for qc in range(2):
                    kmax = 4 if qc == 0 else 8
                    psyA = psy_p.tile([HS + 1, 512], F32, name="psyA", tag="psy")
                    psyB = psy_p.tile([HS + 1, 512], F32, name="psyB", tag="psy")
                    atts = []
                    for kc in range(kmax):
                        atts.append(score_block(ft, 0, qc, kc))
                        atts.append(score_block(ft, 64, qc, kc))
                    if DEBUG_DUMP and ft == 0 and qc == 0:
                        nc.sync.dma_start(dbg["datt"][:], atts[0][:])
                    for kc in range(kmax):
                        n0 = max(kc * P - qc * 512, 0) if USE_NARROW else 0
                        mm(psyA[:, n0:], vt[kc][:, 2 * ft, :],
                           atts[2 * kc][:, n0:], kc == 0, kc == kmax - 1)
                        mm(psyB[:, n0:], vt[kc][:, 2 * ft + 1, :],
                           atts[2 * kc + 1][:, n0:], kc == 0, kc == kmax - 1)
                    for hb, psy in ((0, psyA), (64, psyB)):
                        sl = slice(qc * 512, (qc + 1) * 512)
                        srow = smallp.tile([1, 512], F32, name="srow", tag="srow")
                        nc.vector.tensor_copy(srow[:], psy[HS:HS + 1, :])
                        rsr = smallp.tile([1, 512], F32, name="rsr", tag="rsr")
                        nc.vector.reciprocal_approx_fast(
                            out=rsr[:], in_=srow[:])
                        rb = smallp.tile([P, 512], F32, name="rb", tag="rb")
                        nc.gpsimd.partition_broadcast(rb[:], rsr[0:1, :])
                        if DEBUG_DUMP and ft == 0 and qc == 0 and hb == 0:
                            nc.sync.dma_start(dbg["drb"][:], rb[:])
                        nc.vector.tensor_mul(
                            yt[ft][hb:hb + 64, sl], psy[0:HS, :],
                            rb[hb:hb + 64, :])
                if DEBUG_DUMP and ft == 0:
                    nc.sync.dma_start(dbg["dy"][:], yt[0][:])

        # ---------------- Phase 2: output projection ----------------
        with ExitStack() as dctx:
            outp = dctx.enter_context(tc.tile_pool(name="outp", bufs=4))
            psp_p = dctx.enter_context(tc.tile_pool(name="psp", bufs=3, space="PSUM"))
            for n2 in range(2):
                for sc in range(8):
                    psp = psp_p.tile([P, 512], F32, name="psp", tag="psp")
                    for dc in range(8):
                        mm(psp[:], yt[dc][:, sc * P:(sc + 1) * P],
                           wpt[dc][:, n2 * 512:(n2 + 1) * 512], dc == 0, dc == 7)
                    ot = outp.tile([P, 512], F32, name="ot", tag="ot")
                    nc.scalar.copy(ot[:], psp[:])
                    nc.sync.dma_start(
                        out_d[sc * P:(sc + 1) * P, n2 * 512:(n2 + 1) * 512], ot[:])
    nc.compile()
    return nc


def _prep(inputs):
    w_qkv = np.asarray(inputs["w_qkv"], np.float32)
    w_proj = np.asarray(inputs["w_proj"], np.float32)
    cos = np.asarray(inputs["cos"], np.float32).reshape(S, HS // 2)
    sin = np.asarray(inputs["sin"], np.float32).reshape(S, HS // 2)
    wq, wk, wv = w_qkv[:, 0:D], w_qkv[:, D:2 * D], w_qkv[:, 2 * D:3 * D]
    # (perm/c1/c2 layouts below depend on USE_SHUFFLE)

    cosT = np.ascontiguousarray(cos.T)  # [32, S] freq-major
    sinT = np.ascontiguousarray(sin.T)
    perm = np.empty(D, np.int64)
    c1 = np.empty((P, S), np.float32)
    c2 = np.empty((P, S), np.float32)
    if USE_SHUFFLE:
        # Deinterleave rope pairs so x1/x2 of pair j sit 16 lanes apart
        # inside a 32-lane group: rows [32g:32g+16] = x1 of pairs
        # 16(g%2)+0..15 (features 2j), rows [32g+16:32g+32] = x2 (2j+1).
        for h in range(H):
            b0 = h * HS
            for g in range(2):
                base = b0 + 32 * g
                js = 16 * g + np.arange(16)
                perm[base:base + 16] = b0 + 2 * js
                perm[base + 16:base + 32] = b0 + 2 * js + 1
        for g in range(4):
            hh = g % 2
            c1[32 * g:32 * g + 16] = cosT[16 * hh:16 * hh + 16]
            c1[32 * g + 16:32 * g + 32] = cosT[16 * hh:16 * hh + 16]
            c2[32 * g:32 * g + 16] = -sinT[16 * hh:16 * hh + 16]
            c2[32 * g + 16:32 * g + 32] = sinT[16 * hh:16 * hh + 16]
    else:
        # v1 layout: x1 = rows [0:32] (even features), x2 = rows [32:64]
        for h in range(H):
            b0 = h * HS
            perm[b0:b0 + HS // 2] = b0 + np.arange(0, HS, 2)
            perm[b0 + HS // 2:b0 + HS] = b0 + np.arange(1, HS, 2)
        c1[:] = np.concatenate([cosT, cosT, cosT, cosT], 0)
        c2[:] = np.concatenate([-sinT, sinT, -sinT, sinT], 0)
    wq, wk = wq[:, perm], wk[:, perm]
    scale = np.float32(1.0 / np.sqrt(HS))
    mask = np.triu(np.ones((P, P), np.float32))  # [k, q]: allow q >= k
    common = {
        "wq": np.ascontiguousarray(wq).astype(NPBF),
        "wk": np.ascontiguousarray(wk).astype(NPBF),
        "wv": np.ascontiguousarray(wv).astype(NPBF),
        "wp": np.ascontiguousarray(w_proj).astype(NPBF),
        "c1q": (c1 * scale).astype(NPBF), "c2q": (c2 * scale).astype(NPBF),
        "c1k": c1.astype(NPBF), "c2k": c2.astype(NPBF),
        "mask": mask.astype(NPBF),
        "ones": np.ones((P, H), NPBF),
    }
    return common


LAST_RESULT = None


def kernel(**inputs):
    global LAST_RESULT
    if "nc" not in _CACHE:
        _CACHE["nc"] = _build_nc()
    nc = _CACHE["nc"]
    common = _prep(inputs)
    x = np.asarray(inputs["x"], np.float32)
    in_maps = [dict(common, x=x[b].T.astype(NPBF)) for b in range(B)]
    res = run_bass_kernel_spmd(nc, in_maps, list(range(NCORES)))
    LAST_RESULT = res
    out = np.stack([res.results[i]["out"] for i in range(B)], 0)
    return out.astype(np.float32)
